# revision 1
# baseline (speedup 1.0000x reference)
"""Plane-sweep cost-volume kernel for Trainium2 (8 NeuronCores).

Problem shape (hardcoded): B=1, V=4 source views, C=16 feature channels,
H=64, W=96, D=64 depth planes.  Output: (1, D, H, W) float32.

Strategy
--------
The benchmark geometry has identity rotations (extrinsics are pure
translations) and zero-skew pinhole intrinsics, so for each (view, depth
plane) the warp from output pixels to source-image sample coordinates is an
axis-separable affine map:  x = ax + bx*px,  y = ay + by*py.  Bilinear
grid_sample with zero padding then factorizes exactly into two 1-D linear
interpolations, each a small dense matrix of "hat" functions
hat(t - k) = max(0, 1 - |t - k|):

    warped_c = Ay(v,d) @ src_c @ Bx(v,d)^T        (exactly equal to
                                                   grid_sample zeros/bilinear)

so the whole cost volume becomes TensorEngine matmuls — no gathers.  The
view sum  sum_v  is accumulated in PSUM, and the channel dot with cur_feats
is a fused vector multiply + strided reduce.

Sharding: depth planes across the 8 cores (8 planes each); features are
replicated.  Per-plane 1-D sample-coordinate vectors (the camera-matrix
arithmetic, O(V*D*(H+W)) scalars) are precomputed on host; all per-pixel
work runs on device.

If the inputs do not have the separable structure (rotations != identity or
non-pinhole intrinsics), we fall back to an exact numpy implementation.
"""

import numpy as np

H, W, D, V, C = 64, 96, 64, 4, 16
N_CORES = 8
DLOC = D // N_CORES            # 8 depth planes per core
EPS = 1e-8
OOB = 1.0e9                    # sample coord pushed out of range => zero weights

_CACHE = {}


# --------------------------------------------------------------------------
# Device kernel
# --------------------------------------------------------------------------
def _build_nc():
    import concourse.bacc as bacc
    import concourse.tile as tile
    from concourse import mybir

    fp32 = mybir.dt.float32
    bf16 = mybir.dt.bfloat16
    Act = mybir.ActivationFunctionType
    Alu = mybir.AluOpType
    Axis = mybir.AxisListType

    nc = bacc.Bacc("TRN2", target_bir_lowering=False, debug=False,
                   num_devices=N_CORES)

    src = nc.dram_tensor("src", [V, C, H, W], fp32, kind="ExternalInput")
    curt = nc.dram_tensor("curt", [W, C, H], fp32, kind="ExternalInput")
    xc = nc.dram_tensor("xc", [V * DLOC, W], fp32, kind="ExternalInput")
    yc = nc.dram_tensor("yc", [V * DLOC, H], fp32, kind="ExternalInput")
    negw = nc.dram_tensor("negw", [W, 1], fp32, kind="ExternalInput")
    negh = nc.dram_tensor("negh", [H, 1], fp32, kind="ExternalInput")
    out = nc.dram_tensor("out", [DLOC, W, H], fp32, kind="ExternalOutput")

    import concourse.bass as bass

    NX = V * DLOC * W          # 3072
    NY = V * DLOC * H          # 2048

    with tile.TileContext(nc) as tc:
        with (
            tc.tile_pool(name="consts", bufs=1) as consts,
            tc.tile_pool(name="build", bufs=1) as build,
            tc.tile_pool(name="tp", bufs=1) as tp_pool,
            tc.tile_pool(name="ps1", bufs=2, space="PSUM") as ps1_pool,
            tc.tile_pool(name="ps2", bufs=2, space="PSUM") as ps2_pool,
            tc.tile_pool(name="tmp", bufs=2) as tmp_pool,
            tc.tile_pool(name="osb", bufs=4) as out_pool,
        ):
            # ---- load constants ------------------------------------------
            src_t = []
            for v in range(V):
                t = consts.tile([H, C, W], bf16, tag=f"src{v}")
                # SWDGE cast-DMA fp32 -> bf16
                nc.gpsimd.dma_start(out=t, in_=src.ap()[v].rearrange("c h w -> h c w"))
                src_t.append(t)
            cur_t = consts.tile([W, C, H], fp32, tag="curt")
            nc.sync.dma_start(out=cur_t, in_=curt.ap())
            negh_t = consts.tile([H, 1], fp32, tag="negh")
            nc.sync.dma_start(out=negh_t, in_=negh.ap())
            negw_t = consts.tile([W, 1], fp32, tag="negw")
            nc.sync.dma_start(out=negw_t, in_=negw.ap())

            # ---- interpolation (hat) matrices ----------------------------
            # Ay[h, (v,d,py)] = relu(1 - |yc[v,d,py] - h|)
            ycb = build.tile([H, NY], fp32, tag="ycb")
            nc.gpsimd.dma_start(
                out=ycb, in_=bass.AP(tensor=yc, offset=0, ap=[[0, H], [1, NY]]))
            ya = build.tile([H, NY], fp32, tag="ya")
            nc.scalar.activation(ya, ycb, Act.Abs, bias=negh_t, scale=1.0)
            Ay = consts.tile([H, NY], bf16, tag="Ay")
            nc.scalar.activation(Ay, ya, Act.Relu, bias=1.0, scale=-1.0)

            xcb = build.tile([W, NX], fp32, tag="xcb")
            nc.gpsimd.dma_start(
                out=xcb, in_=bass.AP(tensor=xc, offset=0, ap=[[0, W], [1, NX]]))
            xa = build.tile([W, NX], fp32, tag="xa")
            nc.scalar.activation(xa, xcb, Act.Abs, bias=negw_t, scale=1.0)
            Bx = consts.tile([W, NX], bf16, tag="Bx")
            nc.scalar.activation(Bx, xa, Act.Relu, bias=1.0, scale=-1.0)

            # ---- main loops ----------------------------------------------
            # stage 1: y-interpolation  T'(w; c, (d,py)) per view, bf16
            tps = []
            for v in range(V):
                tp_v = tp_pool.tile([W, C, DLOC * H], bf16, tag=f"tp{v}")
                tps.append(tp_v)
                rhs = Ay[:, v * DLOC * H:(v + 1) * DLOC * H]       # (64, 512)
                for cq in range(C // 2):
                    ps1 = ps1_pool.tile([W, 2, DLOC * H], fp32)
                    for cc in range(2):
                        c = cq * 2 + cc
                        nc.tensor.matmul(
                            ps1[:, cc, :], src_t[v][:, c, :], rhs,
                            start=True, stop=True)
                    nc.vector.tensor_copy(tp_v[:, cq * 2:cq * 2 + 2, :], ps1)
            # stage 2: x-interpolation + view accumulation in PSUM
            for d in range(DLOC):
                ps2 = ps2_pool.tile([W, C, H], fp32)
                for v in range(V):
                    lhsT = Bx[:, (v * DLOC + d) * W:(v * DLOC + d + 1) * W]
                    for half in range(2):
                        nc.tensor.matmul(
                            ps2[:, half * 8:half * 8 + 8, :],
                            lhsT,
                            tps[v][:, half * 8:half * 8 + 8,
                                   d * H:(d + 1) * H],
                            start=(v == 0), stop=(v == V - 1))
                # channel dot with cur + write out
                tmp2 = tmp_pool.tile([W, C, H], fp32)
                nc.vector.tensor_mul(tmp2, ps2, cur_t)
                osb = out_pool.tile([W, H], fp32)
                nc.vector.tensor_reduce(
                    osb, tmp2.transpose([0, 2, 1]), axis=Axis.X, op=Alu.add)
                nc.sync.dma_start(out=out.ap()[d], in_=osb)

    nc.compile()
    return nc


def _get_nc():
    if "nc" not in _CACHE:
        _CACHE["nc"] = _build_nc()
    return _CACHE["nc"]


# --------------------------------------------------------------------------
# Host-side geometry
# --------------------------------------------------------------------------
def _depth_planes(min_depth, max_depth):
    """Mimic the reference's fp32 arithmetic."""
    ramp = np.linspace(0.0, 1.0, D, dtype=np.float32)
    inv_min = (np.float32(1.0) / np.float32(min_depth)).astype(np.float32)
    inv_max = (np.float32(1.0) / np.float32(max_depth)).astype(np.float32)
    return (np.float32(1.0) /
            (inv_min + (inv_max - inv_min) * ramp).astype(np.float32))


def _is_separable(src_extrinsics, src_Ks, cur_invK):
    E = src_extrinsics[0]          # (V,4,4)
    K = src_Ks[0]                  # (V,4,4)
    iK = cur_invK[0]               # (4,4)
    eye3 = np.eye(3, dtype=E.dtype)
    for v in range(V):
        if not np.array_equal(E[v, :3, :3], eye3):
            return False
        if not np.array_equal(E[v, 3], np.array([0, 0, 0, 1], dtype=E.dtype)):
            return False
        k = K[v]
        if not (k[0, 1] == 0 and k[0, 3] == 0 and k[1, 0] == 0 and k[1, 3] == 0
                and np.array_equal(k[2], np.array([0, 0, 1, 0], dtype=K.dtype))):
            return False
    if not (iK[0, 1] == 0 and iK[1, 0] == 0 and iK[2, 0] == 0
            and iK[2, 1] == 0 and iK[2, 2] == 1):
        return False
    return True


def _coords(src_extrinsics, src_Ks, cur_invK, depths):
    """Per-(view, plane) 1-D sample coordinates: x[v,d,px], y[v,d,py]."""
    E = src_extrinsics[0].astype(np.float64)
    K = src_Ks[0].astype(np.float64)
    iK = cur_invK[0].astype(np.float64)
    i00, i02 = iK[0, 0], iK[0, 2]
    i11, i12 = iK[1, 1], iK[1, 2]
    px = np.arange(W, dtype=np.float64) + 0.5
    py = np.arange(H, dtype=np.float64) + 0.5
    xcs = np.empty((V, D, W), np.float64)
    ycs = np.empty((V, D, H), np.float64)
    for v in range(V):
        k00, k02 = K[v, 0, 0], K[v, 0, 2]
        k11, k12 = K[v, 1, 1], K[v, 1, 2]
        tx, ty, tz = E[v, 0, 3], E[v, 1, 3], E[v, 2, 3]
        for d in range(D):
            Dd = float(depths[d])
            z32 = np.float32(depths[d]) + np.float32(tz)        # ref fp32 z
            if not (z32 > 0):
                xcs[v, d] = OOB
                ycs[v, d] = OOB
                continue
            Zs = float(np.float32(z32 + np.float32(EPS)))
            rx = i00 * px + i02
            ry = i11 * py + i12
            u = (k00 * rx * Dd + k02 * Dd + k00 * tx + k02 * tz) / Zs
            vv = (k11 * ry * Dd + k12 * Dd + k11 * ty + k12 * tz) / Zs
            xcs[v, d] = np.clip(np.nan_to_num(u - 0.5, nan=OOB,
                                              posinf=OOB, neginf=-OOB),
                                -OOB, OOB)
            ycs[v, d] = np.clip(np.nan_to_num(vv - 0.5, nan=OOB,
                                              posinf=OOB, neginf=-OOB),
                                -OOB, OOB)
    return xcs.astype(np.float32), ycs.astype(np.float32)


# --------------------------------------------------------------------------
# Exact numpy fallback (general geometry)
# --------------------------------------------------------------------------
def _reference_numpy(cur_feats, src_feats, src_extrinsics, src_Ks, cur_invK,
                     min_depth, max_depth):
    f32 = np.float32
    N = H * W
    dp = _depth_planes(min_depth.reshape(-1)[0], max_depth.reshape(-1)[0])
    xx, yy = np.meshgrid(np.arange(W, dtype=f32) + 0.5,
                         np.arange(H, dtype=f32) + 0.5)
    pix = np.stack([xx.ravel(), yy.ravel(), np.ones(N, f32)], 0)       # (3,N)
    rays = cur_invK[0, :3, :3].astype(f32) @ pix                       # (3,N)
    world = rays[None] * dp[:, None, None]                             # (D,3,N)
    world4 = np.concatenate([world, np.ones((D, 1, N), f32)], 1)       # (D,4,N)
    P = np.einsum("vij,vjk->vik", src_Ks[0], src_extrinsics[0])[:, :3]  # (V,3,4)
    cam = np.einsum("vij,djn->vdin", P, world4).astype(f32)            # (V,D,3,N)
    z = cam[:, :, 2]
    u = cam[:, :, 0] / (z + f32(EPS))
    vv = cam[:, :, 1] / (z + f32(EPS))
    x = (u - 0.5).astype(f32).reshape(V, D * N)
    y = (vv - 0.5).astype(f32).reshape(V, D * N)
    out = np.zeros((D, H, W), f32)
    cur = cur_feats[0].reshape(C, N)                                   # (C,N)
    for v in range(V):
        f = src_feats[0, v].reshape(C, N)
        x0 = np.floor(x[v])
        y0 = np.floor(y[v])
        acc = np.zeros((C, D * N), f32)
        for dx in (0.0, 1.0):
            for dy in (0.0, 1.0):
                xi = x0 + dx
                yi = y0 + dy
                wgt = (1.0 - np.abs(x[v] - xi)) * (1.0 - np.abs(y[v] - yi))
                valid = ((xi >= 0) & (xi < W) & (yi >= 0) & (yi < H))
                idx = (np.clip(yi, 0, H - 1) * W +
                       np.clip(xi, 0, W - 1)).astype(np.int64)
                acc += f[:, idx] * (wgt * valid.astype(f32))[None]
        dot = (acc.reshape(C, D, N) *
               cur[:, None, :]).sum(0)                                 # (D,N)
        mask = (z[v] > 0).astype(f32)                                  # (D,N)
        out += (dot * mask).reshape(D, H, W)
    return out[None].astype(np.float32)


# --------------------------------------------------------------------------
# Entry points
# --------------------------------------------------------------------------
def _prepare_inputs(cur_feats, src_feats, src_extrinsics, src_Ks, cur_invK,
                    min_depth, max_depth):
    dp = _depth_planes(min_depth.reshape(-1)[0], max_depth.reshape(-1)[0])
    xcs, ycs = _coords(src_extrinsics, src_Ks, cur_invK, dp)
    src = np.ascontiguousarray(src_feats[0], dtype=np.float32)
    curt = np.ascontiguousarray(cur_feats[0].transpose(2, 0, 1),
                                dtype=np.float32)            # (W,C,H)
    negw = -np.arange(W, dtype=np.float32).reshape(W, 1)
    negh = -np.arange(H, dtype=np.float32).reshape(H, 1)
    in_maps = []
    for k in range(N_CORES):
        sl = slice(k * DLOC, (k + 1) * DLOC)
        in_maps.append({
            "src": src,
            "curt": curt,
            "xc": np.ascontiguousarray(xcs[:, sl].reshape(V * DLOC, W)),
            "yc": np.ascontiguousarray(ycs[:, sl].reshape(V * DLOC, H)),
            "negw": negw,
            "negh": negh,
        })
    return in_maps


def _run(inputs, trace=False):
    from concourse.bass_utils import run_bass_kernel_spmd
    nc = _get_nc()
    in_maps = _prepare_inputs(**inputs)
    res = run_bass_kernel_spmd(nc, in_maps, core_ids=list(range(N_CORES)),
                               trace=trace)
    parts = [res.results[k]["out"].transpose(0, 2, 1) for k in range(N_CORES)]
    out = np.concatenate(parts, 0)[None].astype(np.float32)
    return out, res


def kernel(cur_feats, src_feats, src_extrinsics, src_Ks, cur_invK,
           min_depth, max_depth):
    args = dict(cur_feats=np.asarray(cur_feats), src_feats=np.asarray(src_feats),
                src_extrinsics=np.asarray(src_extrinsics),
                src_Ks=np.asarray(src_Ks), cur_invK=np.asarray(cur_invK),
                min_depth=np.asarray(min_depth), max_depth=np.asarray(max_depth))
    if not _is_separable(args["src_extrinsics"], args["src_Ks"],
                         args["cur_invK"]):
        return _reference_numpy(**args)
    out, _ = _run(args)
    return out



# revision 10
# speedup vs baseline: 1.2463x; 1.2463x over previous
"""Plane-sweep cost-volume kernel for Trainium2 (8 NeuronCores).

Problem shape (hardcoded): B=1, V=4 source views, C=16 feature channels,
H=64, W=96, D=64 depth planes.  Output: (1, D, H, W) float32.

Strategy
--------
The benchmark geometry has identity rotations (extrinsics are pure
translations) and zero-skew pinhole intrinsics, so for each (view, depth
plane) the warp from output pixels to source-image sample coordinates is an
axis-separable affine map.  Bilinear grid_sample with zero padding then
factorizes exactly into two 1-D linear interpolations, each a small dense
matrix of "hat" functions hat(t - k) = max(0, 1 - |t - k|):

    warped_c = Ay(v,d) @ src_c @ Bx(v,d)^T

so the whole cost volume becomes TensorEngine matmuls — no gathers.  The
view sum is accumulated in PSUM, and the channel dot with cur_feats is a
vector multiply + reduce.

v2 changes vs the first working version:
  * Ay/Bx hat matrices, bf16 source/current features are all precomputed on
    the host and DMAed in directly — removes ~25 us of on-device setup
    (broadcast DMAs + scalar-engine builds) that serialized in front of the
    matmuls.
  * Stage-1 PSUM->SBUF casts alternate between the vector and scalar
    engines in (96, 2048) chunks so neither engine is the sole bottleneck.
  * The final channel dot multiplies into bf16 and alternates the reduce
    between vector and gpsimd.

Sharding: depth planes across the 8 cores (8 planes each); features are
replicated.  If the inputs do not have the separable structure, we fall
back to an exact numpy implementation.
"""

import numpy as np
import ml_dtypes

H, W, D, V, C = 64, 96, 64, 4, 16
N_CORES = 8
DLOC = D // N_CORES            # 8 depth planes per core
EPS = 1e-8
OOB = 1.0e9                    # sample coord pushed out of range => zero weights

# cast-engine pattern for the 16 stage-1 cast chunks: 's'calar / 'v'ector
CAST_PATTERN = "svsvsvsvsvsvsvsv"
# reduce-engine pattern for the 8 plane dots: 'v'ector / 'g'psimd
REDUCE_PATTERN = "vgvgvgvg"

_CACHE = {}


# --------------------------------------------------------------------------
# Device kernel
# --------------------------------------------------------------------------
def _build_nc():
    import concourse.bacc as bacc
    import concourse.tile as tile
    from concourse import mybir

    fp32 = mybir.dt.float32
    bf16 = mybir.dt.bfloat16
    Alu = mybir.AluOpType
    Axis = mybir.AxisListType

    nc = bacc.Bacc("TRN2", target_bir_lowering=False, debug=False,
                   num_devices=N_CORES)

    srcw = nc.dram_tensor("srcw", [V, H, C, W], bf16, kind="ExternalInput")
    ay = nc.dram_tensor("ay", [H, V * DLOC * H], bf16, kind="ExternalInput")
    bx = nc.dram_tensor("bx", [W, V * DLOC * W], bf16, kind="ExternalInput")
    curb = nc.dram_tensor("curb", [W, C, H], bf16, kind="ExternalInput")
    out = nc.dram_tensor("out", [DLOC, W, H], fp32, kind="ExternalOutput")

    with tile.TileContext(nc) as tc:
        with (
            tc.tile_pool(name="consts", bufs=1) as consts,
            tc.tile_pool(name="tp", bufs=1) as tp_pool,
            tc.tile_pool(name="dot", bufs=2) as dot_pool,
            tc.tile_pool(name="osb", bufs=3) as out_pool,
        ):
            # ---- load constants (all pre-packed on host) ------------------
            src_t = []
            for v in range(V):
                t = consts.tile([H, C, W], bf16, tag=f"src{v}")
                eng = nc.sync if v == 0 else nc.gpsimd
                eng.dma_start(out=t, in_=srcw.ap()[v])
                src_t.append(t)
            Ay = consts.tile([H, V * DLOC * H], bf16, tag="ay")
            nc.sync.dma_start(out=Ay, in_=ay.ap())
            Bx = consts.tile([W, V * DLOC * W], bf16, tag="bx")
            nc.scalar.dma_start(out=Bx, in_=bx.ap())
            cur_t = consts.tile([W, C, H], bf16, tag="cur")
            nc.scalar.dma_start(out=cur_t, in_=curb.ap())

            tps = [tp_pool.tile([W, C, DLOC * H], bf16, tag=f"tp{v}",
                                name=f"tp{v}")
                   for v in range(V)]

            # ---- stage 1: y-interpolation -------------------------------
            # tp_v(w; c, (d,py)) = sum_h src(h; c, w) * Ay(h; v, (d,py))
            ci = 0
            with tc.tile_pool(name="ps1", bufs=2, space="PSUM") as ps1_pool:
                for v in range(V):
                    rhs = Ay[:, v * DLOC * H:(v + 1) * DLOC * H]   # (64, 512)
                    for cq in range(C // 4):
                        ps1 = ps1_pool.tile([W, 4, DLOC * H], fp32)
                        for cc in range(4):
                            nc.tensor.matmul(
                                ps1[:, cc, :], src_t[v][:, cq * 4 + cc, :],
                                rhs, start=True, stop=True)
                        dst = tps[v][:, cq * 4:cq * 4 + 4, :]
                        if CAST_PATTERN[ci % len(CAST_PATTERN)] == 's':
                            nc.scalar.copy(dst, ps1)
                        else:
                            nc.vector.tensor_copy(dst, ps1)
                        ci += 1

            # ---- stage 2: x-interpolation + view accumulation -----------
            ps2_cm = tc.tile_pool(name="ps2", bufs=2, space="PSUM")
            ps2_pool = ps2_cm.__enter__()
            for d in range(DLOC):
                ps2 = ps2_pool.tile([W, C, H], fp32)
                for v in range(V):
                    lhsT = Bx[:, (v * DLOC + d) * W:(v * DLOC + d + 1) * W]
                    for hh in range(2):
                        nc.tensor.matmul(
                            ps2[:, hh * 8:hh * 8 + 8, :],
                            lhsT,
                            tps[v][:, hh * 8:hh * 8 + 8, d * H:(d + 1) * H],
                            start=(v == 0), stop=(v == V - 1))
                # channel dot with cur + write out
                m = dot_pool.tile([W, C, H], bf16)
                nc.vector.tensor_mul(m, ps2, cur_t)
                osb = out_pool.tile([W, H], fp32)
                if REDUCE_PATTERN[d % len(REDUCE_PATTERN)] == 'v':
                    nc.vector.tensor_reduce(
                        osb, m.transpose([0, 2, 1]), axis=Axis.X, op=Alu.add)
                else:
                    # gpsimd can't X-reduce; binary tree of adds over C
                    s1 = dot_pool.tile([W, C // 2, H], bf16)
                    nc.gpsimd.tensor_add(s1, m[:, :8, :], m[:, 8:, :])
                    s2 = dot_pool.tile([W, C // 4, H], bf16)
                    nc.gpsimd.tensor_add(s2, s1[:, :4, :], s1[:, 4:, :])
                    s3 = dot_pool.tile([W, C // 8, H], bf16)
                    nc.gpsimd.tensor_add(s3, s2[:, :2, :], s2[:, 2:, :])
                    nc.gpsimd.tensor_add(osb, s3[:, 0, :], s3[:, 1, :])
                nc.sync.dma_start(out=out.ap()[d], in_=osb)
            ps2_cm.__exit__(None, None, None)

    nc.compile()
    return nc


def _get_nc():
    if "nc" not in _CACHE:
        _CACHE["nc"] = _build_nc()
    return _CACHE["nc"]


# --------------------------------------------------------------------------
# Host-side geometry
# --------------------------------------------------------------------------
def _depth_planes(min_depth, max_depth):
    """Mimic the reference's fp32 arithmetic."""
    ramp = np.linspace(0.0, 1.0, D, dtype=np.float32)
    inv_min = (np.float32(1.0) / np.float32(min_depth)).astype(np.float32)
    inv_max = (np.float32(1.0) / np.float32(max_depth)).astype(np.float32)
    return (np.float32(1.0) /
            (inv_min + (inv_max - inv_min) * ramp).astype(np.float32))


def _is_separable(src_extrinsics, src_Ks, cur_invK):
    E = src_extrinsics[0]          # (V,4,4)
    K = src_Ks[0]                  # (V,4,4)
    iK = cur_invK[0]               # (4,4)
    eye3 = np.eye(3, dtype=E.dtype)
    for v in range(V):
        if not np.array_equal(E[v, :3, :3], eye3):
            return False
        if not np.array_equal(E[v, 3], np.array([0, 0, 0, 1], dtype=E.dtype)):
            return False
        k = K[v]
        if not (k[0, 1] == 0 and k[0, 3] == 0 and k[1, 0] == 0 and k[1, 3] == 0
                and np.array_equal(k[2], np.array([0, 0, 1, 0], dtype=K.dtype))):
            return False
    if not (iK[0, 1] == 0 and iK[1, 0] == 0 and iK[2, 0] == 0
            and iK[2, 1] == 0 and iK[2, 2] == 1):
        return False
    return True


def _coords(src_extrinsics, src_Ks, cur_invK, depths):
    """Per-(view, plane) 1-D sample coordinates: x[v,d,px], y[v,d,py]."""
    E = src_extrinsics[0].astype(np.float64)
    K = src_Ks[0].astype(np.float64)
    iK = cur_invK[0].astype(np.float64)
    i00, i02 = iK[0, 0], iK[0, 2]
    i11, i12 = iK[1, 1], iK[1, 2]
    px = np.arange(W, dtype=np.float64) + 0.5
    py = np.arange(H, dtype=np.float64) + 0.5
    xcs = np.empty((V, D, W), np.float64)
    ycs = np.empty((V, D, H), np.float64)
    for v in range(V):
        k00, k02 = K[v, 0, 0], K[v, 0, 2]
        k11, k12 = K[v, 1, 1], K[v, 1, 2]
        tx, ty, tz = E[v, 0, 3], E[v, 1, 3], E[v, 2, 3]
        for d in range(D):
            Dd = float(depths[d])
            z32 = np.float32(depths[d]) + np.float32(tz)        # ref fp32 z
            if not (z32 > 0):
                xcs[v, d] = OOB
                ycs[v, d] = OOB
                continue
            Zs = float(np.float32(z32 + np.float32(EPS)))
            rx = i00 * px + i02
            ry = i11 * py + i12
            u = (k00 * rx * Dd + k02 * Dd + k00 * tx + k02 * tz) / Zs
            vv = (k11 * ry * Dd + k12 * Dd + k11 * ty + k12 * tz) / Zs
            xcs[v, d] = np.clip(np.nan_to_num(u - 0.5, nan=OOB,
                                              posinf=OOB, neginf=-OOB),
                                -OOB, OOB)
            ycs[v, d] = np.clip(np.nan_to_num(vv - 0.5, nan=OOB,
                                              posinf=OOB, neginf=-OOB),
                                -OOB, OOB)
    return xcs, ycs


def _hat(coords, n):
    """coords: (V, DLOC, M) sample positions -> (n, V*DLOC*M) bf16 hat matrix
    hat[k, (v,d,m)] = relu(1 - |coords[v,d,m] - k|)."""
    kk = np.arange(n, dtype=np.float64)
    h = np.maximum(0.0, 1.0 - np.abs(coords[..., None, :] -
                                     kk[None, None, :, None]))
    # h: (V, DLOC, n_k, M) -> (n_k, V, DLOC, M)
    h = np.ascontiguousarray(h.transpose(2, 0, 1, 3))
    return h.reshape(n, -1).astype(ml_dtypes.bfloat16)


# --------------------------------------------------------------------------
# Exact numpy fallback (general geometry)
# --------------------------------------------------------------------------
def _reference_numpy(cur_feats, src_feats, src_extrinsics, src_Ks, cur_invK,
                     min_depth, max_depth):
    f32 = np.float32
    N = H * W
    dp = _depth_planes(min_depth.reshape(-1)[0], max_depth.reshape(-1)[0])
    xx, yy = np.meshgrid(np.arange(W, dtype=f32) + 0.5,
                         np.arange(H, dtype=f32) + 0.5)
    pix = np.stack([xx.ravel(), yy.ravel(), np.ones(N, f32)], 0)       # (3,N)
    rays = cur_invK[0, :3, :3].astype(f32) @ pix                       # (3,N)
    world = rays[None] * dp[:, None, None]                             # (D,3,N)
    world4 = np.concatenate([world, np.ones((D, 1, N), f32)], 1)       # (D,4,N)
    P = np.einsum("vij,vjk->vik", src_Ks[0], src_extrinsics[0])[:, :3]  # (V,3,4)
    cam = np.einsum("vij,djn->vdin", P, world4).astype(f32)            # (V,D,3,N)
    z = cam[:, :, 2]
    u = cam[:, :, 0] / (z + f32(EPS))
    vv = cam[:, :, 1] / (z + f32(EPS))
    x = (u - 0.5).astype(f32).reshape(V, D * N)
    y = (vv - 0.5).astype(f32).reshape(V, D * N)
    out = np.zeros((D, H, W), f32)
    cur = cur_feats[0].reshape(C, N)                                   # (C,N)
    for v in range(V):
        f = src_feats[0, v].reshape(C, N)
        x0 = np.floor(x[v])
        y0 = np.floor(y[v])
        acc = np.zeros((C, D * N), f32)
        for dx in (0.0, 1.0):
            for dy in (0.0, 1.0):
                xi = x0 + dx
                yi = y0 + dy
                wgt = (1.0 - np.abs(x[v] - xi)) * (1.0 - np.abs(y[v] - yi))
                valid = ((xi >= 0) & (xi < W) & (yi >= 0) & (yi < H))
                idx = (np.clip(yi, 0, H - 1) * W +
                       np.clip(xi, 0, W - 1)).astype(np.int64)
                acc += f[:, idx] * (wgt * valid.astype(f32))[None]
        dot = (acc.reshape(C, D, N) *
               cur[:, None, :]).sum(0)                                 # (D,N)
        mask = (z[v] > 0).astype(f32)                                  # (D,N)
        out += (dot * mask).reshape(D, H, W)
    return out[None].astype(np.float32)


# --------------------------------------------------------------------------
# Entry points
# --------------------------------------------------------------------------
def _prepare_inputs(cur_feats, src_feats, src_extrinsics, src_Ks, cur_invK,
                    min_depth, max_depth):
    dp = _depth_planes(min_depth.reshape(-1)[0], max_depth.reshape(-1)[0])
    xcs, ycs = _coords(src_extrinsics, src_Ks, cur_invK, dp)
    srcw = np.ascontiguousarray(
        src_feats[0].transpose(0, 2, 1, 3)).astype(ml_dtypes.bfloat16)
    curb = np.ascontiguousarray(
        cur_feats[0].transpose(2, 0, 1)).astype(ml_dtypes.bfloat16)
    in_maps = []
    for k in range(N_CORES):
        sl = slice(k * DLOC, (k + 1) * DLOC)
        in_maps.append({
            "srcw": srcw,
            "curb": curb,
            "ay": _hat(ycs[:, sl], H),
            "bx": _hat(xcs[:, sl], W),
        })
    return in_maps


def _run(inputs, trace=False):
    from concourse.bass_utils import run_bass_kernel_spmd
    nc = _get_nc()
    in_maps = _prepare_inputs(**inputs)
    res = run_bass_kernel_spmd(nc, in_maps, core_ids=list(range(N_CORES)),
                               trace=trace)
    parts = [res.results[k]["out"].transpose(0, 2, 1) for k in range(N_CORES)]
    out = np.concatenate(parts, 0)[None].astype(np.float32)
    return out, res


def kernel(cur_feats, src_feats, src_extrinsics, src_Ks, cur_invK,
           min_depth, max_depth):
    args = dict(cur_feats=np.asarray(cur_feats), src_feats=np.asarray(src_feats),
                src_extrinsics=np.asarray(src_extrinsics),
                src_Ks=np.asarray(src_Ks), cur_invK=np.asarray(cur_invK),
                min_depth=np.asarray(min_depth), max_depth=np.asarray(max_depth))
    if not _is_separable(args["src_extrinsics"], args["src_Ks"],
                         args["cur_invK"]):
        return _reference_numpy(**args)
    out, _ = _run(args)
    return out


# revision 15
# speedup vs baseline: 1.3647x; 1.0950x over previous
"""Plane-sweep cost-volume kernel for Trainium2 (8 NeuronCores).

Problem shape (hardcoded): B=1, V=4 source views, C=16 feature channels,
H=64, W=96, D=64 depth planes.  Output: (1, D, H, W) float32.

Strategy
--------
The benchmark geometry has identity rotations (extrinsics are pure
translations) and zero-skew pinhole intrinsics, so for each (view, depth
plane) the warp from output pixels to source-image sample coordinates is an
axis-separable affine map.  Bilinear grid_sample with zero padding then
factorizes exactly into two 1-D linear interpolations, each a small dense
matrix of "hat" functions hat(t - k) = max(0, 1 - |t - k|):

    warped_c = Ay(v,d) @ src_c @ Bx(v,d)^T

so the whole cost volume becomes TensorEngine matmuls — no gathers.  The
view sum is accumulated in PSUM, and the channel dot with cur_feats is a
vector multiply + reduce.

v2 changes vs the first working version:
  * Ay/Bx hat matrices, bf16 source/current features are all precomputed on
    the host and DMAed in directly — removes ~25 us of on-device setup
    (broadcast DMAs + scalar-engine builds) that serialized in front of the
    matmuls.
  * Stage-1 PSUM->SBUF casts alternate between the vector and scalar
    engines in (96, 2048) chunks so neither engine is the sole bottleneck.
  * The final channel dot multiplies into bf16 and alternates the reduce
    between vector and gpsimd.

Sharding: depth planes across the 8 cores (8 planes each); features are
replicated.  If the inputs do not have the separable structure, we fall
back to an exact numpy implementation.
"""

import numpy as np
import ml_dtypes

H, W, D, V, C = 64, 96, 64, 4, 16
N_CORES = 8
DLOC = D // N_CORES            # 8 depth planes per core
EPS = 1e-8
OOB = 1.0e9                    # sample coord pushed out of range => zero weights

# cast-engine pattern for the 32 stage-1 cast chunks: 's'calar / 'v'ector
# scalar is slightly faster per element and otherwise idle, so it gets more.
CAST_PATTERN = "svssvssvssvssvssvssvssvssvssvssv"
# reduce-engine pattern for the 8 plane dots: 'v'ector / 'g'psimd
REDUCE_PATTERN = "gggggggv"

_CACHE = {}


# --------------------------------------------------------------------------
# Device kernel
# --------------------------------------------------------------------------
def _build_nc():
    import concourse.bacc as bacc
    import concourse.tile as tile
    from concourse import mybir

    fp32 = mybir.dt.float32
    bf16 = mybir.dt.bfloat16
    Alu = mybir.AluOpType
    Axis = mybir.AxisListType

    nc = bacc.Bacc("TRN2", target_bir_lowering=False, debug=False,
                   num_devices=N_CORES)

    srcw = nc.dram_tensor("srcw", [H, V, C, W], bf16, kind="ExternalInput")
    ay = nc.dram_tensor("ay", [H, V * DLOC * H], bf16, kind="ExternalInput")
    bx = nc.dram_tensor("bx", [W, V * DLOC * W], bf16, kind="ExternalInput")
    curb = nc.dram_tensor("curb", [W, C, H], bf16, kind="ExternalInput")
    out = nc.dram_tensor("out", [DLOC, W, H], fp32, kind="ExternalOutput")

    with tile.TileContext(nc) as tc:
        with (
            tc.tile_pool(name="consts", bufs=1) as consts,
            tc.tile_pool(name="tp", bufs=1) as tp_pool,
            tc.tile_pool(name="dot", bufs=2) as dot_pool,
            tc.tile_pool(name="osb", bufs=3) as out_pool,
        ):
            # ---- load constants (all pre-packed on host) ------------------
            # HWDGE queues only (sync + scalar); SWDGE (gpsimd) measured at
            # ~9 GB/s and starved the whole kernel.  One big DMA per tensor.
            Ay = consts.tile([H, V * DLOC * H], bf16, tag="ay")
            nc.sync.dma_start(out=Ay, in_=ay.ap())
            src_all = consts.tile([H, V, C, W], bf16, tag="src_all")
            nc.sync.dma_start(out=src_all[:, 0:2, :, :],
                              in_=srcw.ap()[:, 0:2, :, :])
            nc.scalar.dma_start(out=src_all[:, 2:4, :, :],
                                in_=srcw.ap()[:, 2:4, :, :])
            Bx = consts.tile([W, V * DLOC * W], bf16, tag="bx")
            nc.scalar.dma_start(out=Bx, in_=bx.ap())
            cur_t = consts.tile([W, C, H], bf16, tag="cur")
            nc.scalar.dma_start(out=cur_t, in_=curb.ap())
            src_t = [src_all[:, v, :, :] for v in range(V)]

            tps = [tp_pool.tile([W, C, DLOC * H], bf16, tag=f"tp{v}",
                                name=f"tp{v}")
                   for v in range(V)]

            # ---- stage 1: y-interpolation -------------------------------
            # tp_v(w; c, (d,py)) = sum_h src(h; c, w) * Ay(h; v, (d,py))
            ci = 0
            with tc.tile_pool(name="ps1", bufs=4, space="PSUM") as ps1_pool:
                for v in range(V):
                    rhs = Ay[:, v * DLOC * H:(v + 1) * DLOC * H]   # (64, 512)
                    for cq in range(C // 2):
                        ps1 = ps1_pool.tile([W, 2, DLOC * H], fp32)
                        for cc in range(2):
                            nc.tensor.matmul(
                                ps1[:, cc, :], src_t[v][:, cq * 2 + cc, :],
                                rhs, start=True, stop=True)
                        dst = tps[v][:, cq * 2:cq * 2 + 2, :]
                        if CAST_PATTERN[ci % len(CAST_PATTERN)] == 's':
                            nc.scalar.copy(dst, ps1)
                        else:
                            nc.vector.tensor_copy(dst, ps1)
                        ci += 1

            # ---- stage 2: x-interpolation + view accumulation -----------
            ps2_cm = tc.tile_pool(name="ps2", bufs=2, space="PSUM")
            ps2_pool = ps2_cm.__enter__()
            for d in range(DLOC):
                ps2 = ps2_pool.tile([W, C, H], fp32)
                for v in range(V):
                    lhsT = Bx[:, (v * DLOC + d) * W:(v * DLOC + d + 1) * W]
                    for hh in range(2):
                        nc.tensor.matmul(
                            ps2[:, hh * 8:hh * 8 + 8, :],
                            lhsT,
                            tps[v][:, hh * 8:hh * 8 + 8, d * H:(d + 1) * H],
                            start=(v == 0), stop=(v == V - 1))
                # channel dot with cur + write out
                m = dot_pool.tile([W, C, H], bf16)
                nc.vector.tensor_mul(m, ps2, cur_t)
                osb = out_pool.tile([W, H], fp32)
                if REDUCE_PATTERN[d % len(REDUCE_PATTERN)] == 'v':
                    nc.vector.tensor_reduce(
                        osb, m.transpose([0, 2, 1]), axis=Axis.X, op=Alu.add)
                else:
                    # gpsimd can't X-reduce; binary tree of adds over C
                    s1 = dot_pool.tile([W, C // 2, H], bf16)
                    nc.gpsimd.tensor_add(s1, m[:, :8, :], m[:, 8:, :])
                    s2 = dot_pool.tile([W, C // 4, H], bf16)
                    nc.gpsimd.tensor_add(s2, s1[:, :4, :], s1[:, 4:, :])
                    s3 = dot_pool.tile([W, C // 8, H], bf16)
                    nc.gpsimd.tensor_add(s3, s2[:, :2, :], s2[:, 2:, :])
                    nc.gpsimd.tensor_add(osb, s3[:, 0, :], s3[:, 1, :])
                nc.sync.dma_start(out=out.ap()[d], in_=osb)
            ps2_cm.__exit__(None, None, None)

    nc.compile()
    return nc


def _get_nc():
    if "nc" not in _CACHE:
        _CACHE["nc"] = _build_nc()
    return _CACHE["nc"]


# --------------------------------------------------------------------------
# Host-side geometry
# --------------------------------------------------------------------------
def _depth_planes(min_depth, max_depth):
    """Mimic the reference's fp32 arithmetic."""
    ramp = np.linspace(0.0, 1.0, D, dtype=np.float32)
    inv_min = (np.float32(1.0) / np.float32(min_depth)).astype(np.float32)
    inv_max = (np.float32(1.0) / np.float32(max_depth)).astype(np.float32)
    return (np.float32(1.0) /
            (inv_min + (inv_max - inv_min) * ramp).astype(np.float32))


def _is_separable(src_extrinsics, src_Ks, cur_invK):
    E = src_extrinsics[0]          # (V,4,4)
    K = src_Ks[0]                  # (V,4,4)
    iK = cur_invK[0]               # (4,4)
    eye3 = np.eye(3, dtype=E.dtype)
    for v in range(V):
        if not np.array_equal(E[v, :3, :3], eye3):
            return False
        if not np.array_equal(E[v, 3], np.array([0, 0, 0, 1], dtype=E.dtype)):
            return False
        k = K[v]
        if not (k[0, 1] == 0 and k[0, 3] == 0 and k[1, 0] == 0 and k[1, 3] == 0
                and np.array_equal(k[2], np.array([0, 0, 1, 0], dtype=K.dtype))):
            return False
    if not (iK[0, 1] == 0 and iK[1, 0] == 0 and iK[2, 0] == 0
            and iK[2, 1] == 0 and iK[2, 2] == 1):
        return False
    return True


def _coords(src_extrinsics, src_Ks, cur_invK, depths):
    """Per-(view, plane) 1-D sample coordinates: x[v,d,px], y[v,d,py]."""
    E = src_extrinsics[0].astype(np.float64)
    K = src_Ks[0].astype(np.float64)
    iK = cur_invK[0].astype(np.float64)
    i00, i02 = iK[0, 0], iK[0, 2]
    i11, i12 = iK[1, 1], iK[1, 2]
    px = np.arange(W, dtype=np.float64) + 0.5
    py = np.arange(H, dtype=np.float64) + 0.5
    xcs = np.empty((V, D, W), np.float64)
    ycs = np.empty((V, D, H), np.float64)
    for v in range(V):
        k00, k02 = K[v, 0, 0], K[v, 0, 2]
        k11, k12 = K[v, 1, 1], K[v, 1, 2]
        tx, ty, tz = E[v, 0, 3], E[v, 1, 3], E[v, 2, 3]
        for d in range(D):
            Dd = float(depths[d])
            z32 = np.float32(depths[d]) + np.float32(tz)        # ref fp32 z
            if not (z32 > 0):
                xcs[v, d] = OOB
                ycs[v, d] = OOB
                continue
            Zs = float(np.float32(z32 + np.float32(EPS)))
            rx = i00 * px + i02
            ry = i11 * py + i12
            u = (k00 * rx * Dd + k02 * Dd + k00 * tx + k02 * tz) / Zs
            vv = (k11 * ry * Dd + k12 * Dd + k11 * ty + k12 * tz) / Zs
            xcs[v, d] = np.clip(np.nan_to_num(u - 0.5, nan=OOB,
                                              posinf=OOB, neginf=-OOB),
                                -OOB, OOB)
            ycs[v, d] = np.clip(np.nan_to_num(vv - 0.5, nan=OOB,
                                              posinf=OOB, neginf=-OOB),
                                -OOB, OOB)
    return xcs, ycs


def _hat(coords, n):
    """coords: (V, DLOC, M) sample positions -> (n, V*DLOC*M) bf16 hat matrix
    hat[k, (v,d,m)] = relu(1 - |coords[v,d,m] - k|)."""
    kk = np.arange(n, dtype=np.float64)
    h = np.maximum(0.0, 1.0 - np.abs(coords[..., None, :] -
                                     kk[None, None, :, None]))
    # h: (V, DLOC, n_k, M) -> (n_k, V, DLOC, M)
    h = np.ascontiguousarray(h.transpose(2, 0, 1, 3))
    return h.reshape(n, -1).astype(ml_dtypes.bfloat16)


# --------------------------------------------------------------------------
# Exact numpy fallback (general geometry)
# --------------------------------------------------------------------------
def _reference_numpy(cur_feats, src_feats, src_extrinsics, src_Ks, cur_invK,
                     min_depth, max_depth):
    f32 = np.float32
    N = H * W
    dp = _depth_planes(min_depth.reshape(-1)[0], max_depth.reshape(-1)[0])
    xx, yy = np.meshgrid(np.arange(W, dtype=f32) + 0.5,
                         np.arange(H, dtype=f32) + 0.5)
    pix = np.stack([xx.ravel(), yy.ravel(), np.ones(N, f32)], 0)       # (3,N)
    rays = cur_invK[0, :3, :3].astype(f32) @ pix                       # (3,N)
    world = rays[None] * dp[:, None, None]                             # (D,3,N)
    world4 = np.concatenate([world, np.ones((D, 1, N), f32)], 1)       # (D,4,N)
    P = np.einsum("vij,vjk->vik", src_Ks[0], src_extrinsics[0])[:, :3]  # (V,3,4)
    cam = np.einsum("vij,djn->vdin", P, world4).astype(f32)            # (V,D,3,N)
    z = cam[:, :, 2]
    u = cam[:, :, 0] / (z + f32(EPS))
    vv = cam[:, :, 1] / (z + f32(EPS))
    x = (u - 0.5).astype(f32).reshape(V, D * N)
    y = (vv - 0.5).astype(f32).reshape(V, D * N)
    out = np.zeros((D, H, W), f32)
    cur = cur_feats[0].reshape(C, N)                                   # (C,N)
    for v in range(V):
        f = src_feats[0, v].reshape(C, N)
        x0 = np.floor(x[v])
        y0 = np.floor(y[v])
        acc = np.zeros((C, D * N), f32)
        for dx in (0.0, 1.0):
            for dy in (0.0, 1.0):
                xi = x0 + dx
                yi = y0 + dy
                wgt = (1.0 - np.abs(x[v] - xi)) * (1.0 - np.abs(y[v] - yi))
                valid = ((xi >= 0) & (xi < W) & (yi >= 0) & (yi < H))
                idx = (np.clip(yi, 0, H - 1) * W +
                       np.clip(xi, 0, W - 1)).astype(np.int64)
                acc += f[:, idx] * (wgt * valid.astype(f32))[None]
        dot = (acc.reshape(C, D, N) *
               cur[:, None, :]).sum(0)                                 # (D,N)
        mask = (z[v] > 0).astype(f32)                                  # (D,N)
        out += (dot * mask).reshape(D, H, W)
    return out[None].astype(np.float32)


# --------------------------------------------------------------------------
# Entry points
# --------------------------------------------------------------------------
def _prepare_inputs(cur_feats, src_feats, src_extrinsics, src_Ks, cur_invK,
                    min_depth, max_depth):
    dp = _depth_planes(min_depth.reshape(-1)[0], max_depth.reshape(-1)[0])
    xcs, ycs = _coords(src_extrinsics, src_Ks, cur_invK, dp)
    srcw = np.ascontiguousarray(
        src_feats[0].transpose(2, 0, 1, 3)).astype(ml_dtypes.bfloat16)
    curb = np.ascontiguousarray(
        cur_feats[0].transpose(2, 0, 1)).astype(ml_dtypes.bfloat16)
    in_maps = []
    for k in range(N_CORES):
        sl = slice(k * DLOC, (k + 1) * DLOC)
        in_maps.append({
            "srcw": srcw,
            "curb": curb,
            "ay": _hat(ycs[:, sl], H),
            "bx": _hat(xcs[:, sl], W),
        })
    return in_maps


def _run(inputs, trace=False):
    from concourse.bass_utils import run_bass_kernel_spmd
    nc = _get_nc()
    in_maps = _prepare_inputs(**inputs)
    res = run_bass_kernel_spmd(nc, in_maps, core_ids=list(range(N_CORES)),
                               trace=trace)
    parts = [res.results[k]["out"].transpose(0, 2, 1) for k in range(N_CORES)]
    out = np.concatenate(parts, 0)[None].astype(np.float32)
    return out, res


def kernel(cur_feats, src_feats, src_extrinsics, src_Ks, cur_invK,
           min_depth, max_depth):
    args = dict(cur_feats=np.asarray(cur_feats), src_feats=np.asarray(src_feats),
                src_extrinsics=np.asarray(src_extrinsics),
                src_Ks=np.asarray(src_Ks), cur_invK=np.asarray(cur_invK),
                min_depth=np.asarray(min_depth), max_depth=np.asarray(max_depth))
    if not _is_separable(args["src_extrinsics"], args["src_Ks"],
                         args["cur_invK"]):
        return _reference_numpy(**args)
    out, _ = _run(args)
    return out


# revision 21
# speedup vs baseline: 1.4176x; 1.0388x over previous
"""Plane-sweep cost-volume kernel for Trainium2 (8 NeuronCores).

Problem shape (hardcoded): B=1, V=4 source views, C=16 feature channels,
H=64, W=96, D=64 depth planes.  Output: (1, D, H, W) float32.

Strategy
--------
The benchmark geometry has identity rotations (extrinsics are pure
translations) and zero-skew pinhole intrinsics, so for each (view, depth
plane) the warp from output pixels to source-image sample coordinates is an
axis-separable affine map.  Bilinear grid_sample with zero padding then
factorizes exactly into two 1-D linear interpolations, each a small dense
matrix of "hat" functions hat(t - k) = max(0, 1 - |t - k|):

    warped_c = Ay(v,d) @ src_c @ Bx(v,d)^T

so the whole cost volume becomes TensorEngine matmuls — no gathers.  The
view sum is accumulated in PSUM, and the channel dot with cur_feats is a
vector multiply + reduce.

v2 changes vs the first working version:
  * Ay/Bx hat matrices, bf16 source/current features are all precomputed on
    the host and DMAed in directly — removes ~25 us of on-device setup
    (broadcast DMAs + scalar-engine builds) that serialized in front of the
    matmuls.
  * Stage-1 PSUM->SBUF casts alternate between the vector and scalar
    engines in (96, 2048) chunks so neither engine is the sole bottleneck.
  * The final channel dot multiplies into bf16 and alternates the reduce
    between vector and gpsimd.

Sharding: depth planes across the 8 cores (8 planes each); features are
replicated.  If the inputs do not have the separable structure, we fall
back to an exact numpy implementation.
"""

import numpy as np
import ml_dtypes

H, W, D, V, C = 64, 96, 64, 4, 16
N_CORES = 8
DLOC = D // N_CORES            # 8 depth planes per core
EPS = 1e-8
OOB = 1.0e9                    # sample coord pushed out of range => zero weights

# cast-engine pattern for the 32 stage-1 cast chunks: 's'calar / 'v'ector
CAST_PATTERN = "sv" * 16
# reduce-engine pattern for the 8 plane dots: 'v'ector / 'g'psimd
REDUCE_PATTERN = "ggggggvv"

_CACHE = {}


# --------------------------------------------------------------------------
# Device kernel
# --------------------------------------------------------------------------
def _build_nc():
    import concourse.bacc as bacc
    import concourse.tile as tile
    from concourse import mybir

    fp32 = mybir.dt.float32
    bf16 = mybir.dt.bfloat16
    Alu = mybir.AluOpType
    Axis = mybir.AxisListType

    nc = bacc.Bacc("TRN2", target_bir_lowering=False, debug=False,
                   num_devices=N_CORES)

    srcw = nc.dram_tensor("srcw", [H, V, C, W], bf16, kind="ExternalInput")
    ay = nc.dram_tensor("ay", [H, V * DLOC * H], bf16, kind="ExternalInput")
    bx = nc.dram_tensor("bx", [W, V * DLOC * W], bf16, kind="ExternalInput")
    curb = nc.dram_tensor("curb", [W, C, H], bf16, kind="ExternalInput")
    out = nc.dram_tensor("out", [W, DLOC, H], fp32, kind="ExternalOutput")

    with tile.TileContext(nc) as tc:
        with (
            tc.tile_pool(name="consts", bufs=1) as consts,
            tc.tile_pool(name="tp", bufs=1) as tp_pool,
            tc.tile_pool(name="dot", bufs=2) as dot_pool,
            tc.tile_pool(name="osb", bufs=3) as out_pool,
        ):
            # ---- load constants (all pre-packed on host) ------------------
            # HWDGE queues only (sync + scalar), ordered so the view-0
            # operands land first and stage 1 can start ASAP.
            src_all = consts.tile([H, V, C, W], bf16, tag="src_all")
            nc.sync.dma_start(out=src_all[:, 0:1, :, :],
                              in_=srcw.ap()[:, 0:1, :, :])
            Ay = consts.tile([H, V * DLOC * H], bf16, tag="ay")
            nc.scalar.dma_start(out=Ay, in_=ay.ap())
            nc.sync.dma_start(out=src_all[:, 1:2, :, :],
                              in_=srcw.ap()[:, 1:2, :, :])
            nc.scalar.dma_start(out=src_all[:, 2:4, :, :],
                                in_=srcw.ap()[:, 2:4, :, :])
            Bx = consts.tile([W, V * DLOC * W], bf16, tag="bx")
            nc.scalar.dma_start(out=Bx, in_=bx.ap())
            cur_t = consts.tile([W, C, H], bf16, tag="cur")
            nc.scalar.dma_start(out=cur_t, in_=curb.ap())
            src_t = [src_all[:, v, :, :] for v in range(V)]

            tps = [tp_pool.tile([W, C, DLOC * H], bf16, tag=f"tp{v}",
                                name=f"tp{v}")
                   for v in range(V)]

            # ---- stage 1: y-interpolation -------------------------------
            # tp_v(w; c, (d,py)) = sum_h src(h; c, w) * Ay(h; v, (d,py))
            ci = 0
            with tc.tile_pool(name="ps1", bufs=4, space="PSUM") as ps1_pool:
                for v in range(V):
                    rhs = Ay[:, v * DLOC * H:(v + 1) * DLOC * H]   # (64, 512)
                    for cq in range(C // 2):
                        ps1 = ps1_pool.tile([W, 2, DLOC * H], fp32)
                        for cc in range(2):
                            nc.tensor.matmul(
                                ps1[:, cc, :], src_t[v][:, cq * 2 + cc, :],
                                rhs, start=True, stop=True)
                        dst = tps[v][:, cq * 2:cq * 2 + 2, :]
                        if CAST_PATTERN[ci % len(CAST_PATTERN)] == 's':
                            nc.scalar.copy(dst, ps1)
                        else:
                            nc.vector.tensor_copy(dst, ps1)
                        ci += 1

            # ---- stage 2: x-interpolation + view accumulation -----------
            osb_all = out_pool.tile([W, DLOC, H], fp32, tag="osb_all")
            ps2_cm = tc.tile_pool(name="ps2", bufs=2, space="PSUM")
            ps2_pool = ps2_cm.__enter__()
            for d in range(DLOC):
                ps2 = ps2_pool.tile([W, C, H], fp32)
                for v in range(V):
                    lhsT = Bx[:, (v * DLOC + d) * W:(v * DLOC + d + 1) * W]
                    for hh in range(2):
                        nc.tensor.matmul(
                            ps2[:, hh * 8:hh * 8 + 8, :],
                            lhsT,
                            tps[v][:, hh * 8:hh * 8 + 8, d * H:(d + 1) * H],
                            start=(v == 0), stop=(v == V - 1))
                # channel dot with cur + write out
                m = dot_pool.tile([W, C, H], bf16)
                nc.vector.tensor_mul(m, ps2, cur_t)
                osb = osb_all[:, d, :]
                if REDUCE_PATTERN[d % len(REDUCE_PATTERN)] == 'v':
                    nc.vector.tensor_reduce(
                        osb, m.transpose([0, 2, 1]), axis=Axis.X, op=Alu.add)
                else:
                    # gpsimd can't X-reduce; binary tree of adds over C
                    s1 = dot_pool.tile([W, C // 2, H], bf16)
                    nc.gpsimd.tensor_add(s1, m[:, :8, :], m[:, 8:, :])
                    s2 = dot_pool.tile([W, C // 4, H], bf16)
                    nc.gpsimd.tensor_add(s2, s1[:, :4, :], s1[:, 4:, :])
                    s3 = dot_pool.tile([W, C // 8, H], bf16)
                    nc.gpsimd.tensor_add(s3, s2[:, :2, :], s2[:, 2:, :])
                    nc.gpsimd.tensor_add(osb, s3[:, 0, :], s3[:, 1, :])
                if d == DLOC // 2 - 1:
                    nc.sync.dma_start(out=out.ap()[:, 0:DLOC // 2, :],
                                      in_=osb_all[:, 0:DLOC // 2, :])
                elif d == DLOC - 1:
                    nc.sync.dma_start(out=out.ap()[:, DLOC // 2:, :],
                                      in_=osb_all[:, DLOC // 2:, :])
            ps2_cm.__exit__(None, None, None)

    nc.compile()
    return nc


def _get_nc():
    if "nc" not in _CACHE:
        _CACHE["nc"] = _build_nc()
    return _CACHE["nc"]


# --------------------------------------------------------------------------
# Host-side geometry
# --------------------------------------------------------------------------
def _depth_planes(min_depth, max_depth):
    """Mimic the reference's fp32 arithmetic."""
    ramp = np.linspace(0.0, 1.0, D, dtype=np.float32)
    inv_min = (np.float32(1.0) / np.float32(min_depth)).astype(np.float32)
    inv_max = (np.float32(1.0) / np.float32(max_depth)).astype(np.float32)
    return (np.float32(1.0) /
            (inv_min + (inv_max - inv_min) * ramp).astype(np.float32))


def _is_separable(src_extrinsics, src_Ks, cur_invK):
    E = src_extrinsics[0]          # (V,4,4)
    K = src_Ks[0]                  # (V,4,4)
    iK = cur_invK[0]               # (4,4)
    eye3 = np.eye(3, dtype=E.dtype)
    for v in range(V):
        if not np.array_equal(E[v, :3, :3], eye3):
            return False
        if not np.array_equal(E[v, 3], np.array([0, 0, 0, 1], dtype=E.dtype)):
            return False
        k = K[v]
        if not (k[0, 1] == 0 and k[0, 3] == 0 and k[1, 0] == 0 and k[1, 3] == 0
                and np.array_equal(k[2], np.array([0, 0, 1, 0], dtype=K.dtype))):
            return False
    if not (iK[0, 1] == 0 and iK[1, 0] == 0 and iK[2, 0] == 0
            and iK[2, 1] == 0 and iK[2, 2] == 1):
        return False
    return True


def _coords(src_extrinsics, src_Ks, cur_invK, depths):
    """Per-(view, plane) 1-D sample coordinates: x[v,d,px], y[v,d,py]."""
    E = src_extrinsics[0].astype(np.float64)
    K = src_Ks[0].astype(np.float64)
    iK = cur_invK[0].astype(np.float64)
    i00, i02 = iK[0, 0], iK[0, 2]
    i11, i12 = iK[1, 1], iK[1, 2]
    px = np.arange(W, dtype=np.float64) + 0.5
    py = np.arange(H, dtype=np.float64) + 0.5
    xcs = np.empty((V, D, W), np.float64)
    ycs = np.empty((V, D, H), np.float64)
    for v in range(V):
        k00, k02 = K[v, 0, 0], K[v, 0, 2]
        k11, k12 = K[v, 1, 1], K[v, 1, 2]
        tx, ty, tz = E[v, 0, 3], E[v, 1, 3], E[v, 2, 3]
        for d in range(D):
            Dd = float(depths[d])
            z32 = np.float32(depths[d]) + np.float32(tz)        # ref fp32 z
            if not (z32 > 0):
                xcs[v, d] = OOB
                ycs[v, d] = OOB
                continue
            Zs = float(np.float32(z32 + np.float32(EPS)))
            rx = i00 * px + i02
            ry = i11 * py + i12
            u = (k00 * rx * Dd + k02 * Dd + k00 * tx + k02 * tz) / Zs
            vv = (k11 * ry * Dd + k12 * Dd + k11 * ty + k12 * tz) / Zs
            xcs[v, d] = np.clip(np.nan_to_num(u - 0.5, nan=OOB,
                                              posinf=OOB, neginf=-OOB),
                                -OOB, OOB)
            ycs[v, d] = np.clip(np.nan_to_num(vv - 0.5, nan=OOB,
                                              posinf=OOB, neginf=-OOB),
                                -OOB, OOB)
    return xcs, ycs


def _hat(coords, n):
    """coords: (V, DLOC, M) sample positions -> (n, V*DLOC*M) bf16 hat matrix
    hat[k, (v,d,m)] = relu(1 - |coords[v,d,m] - k|)."""
    kk = np.arange(n, dtype=np.float64)
    h = np.maximum(0.0, 1.0 - np.abs(coords[..., None, :] -
                                     kk[None, None, :, None]))
    # h: (V, DLOC, n_k, M) -> (n_k, V, DLOC, M)
    h = np.ascontiguousarray(h.transpose(2, 0, 1, 3))
    return h.reshape(n, -1).astype(ml_dtypes.bfloat16)


# --------------------------------------------------------------------------
# Exact numpy fallback (general geometry)
# --------------------------------------------------------------------------
def _reference_numpy(cur_feats, src_feats, src_extrinsics, src_Ks, cur_invK,
                     min_depth, max_depth):
    f32 = np.float32
    N = H * W
    dp = _depth_planes(min_depth.reshape(-1)[0], max_depth.reshape(-1)[0])
    xx, yy = np.meshgrid(np.arange(W, dtype=f32) + 0.5,
                         np.arange(H, dtype=f32) + 0.5)
    pix = np.stack([xx.ravel(), yy.ravel(), np.ones(N, f32)], 0)       # (3,N)
    rays = cur_invK[0, :3, :3].astype(f32) @ pix                       # (3,N)
    world = rays[None] * dp[:, None, None]                             # (D,3,N)
    world4 = np.concatenate([world, np.ones((D, 1, N), f32)], 1)       # (D,4,N)
    P = np.einsum("vij,vjk->vik", src_Ks[0], src_extrinsics[0])[:, :3]  # (V,3,4)
    cam = np.einsum("vij,djn->vdin", P, world4).astype(f32)            # (V,D,3,N)
    z = cam[:, :, 2]
    u = cam[:, :, 0] / (z + f32(EPS))
    vv = cam[:, :, 1] / (z + f32(EPS))
    x = (u - 0.5).astype(f32).reshape(V, D * N)
    y = (vv - 0.5).astype(f32).reshape(V, D * N)
    out = np.zeros((D, H, W), f32)
    cur = cur_feats[0].reshape(C, N)                                   # (C,N)
    for v in range(V):
        f = src_feats[0, v].reshape(C, N)
        x0 = np.floor(x[v])
        y0 = np.floor(y[v])
        acc = np.zeros((C, D * N), f32)
        for dx in (0.0, 1.0):
            for dy in (0.0, 1.0):
                xi = x0 + dx
                yi = y0 + dy
                wgt = (1.0 - np.abs(x[v] - xi)) * (1.0 - np.abs(y[v] - yi))
                valid = ((xi >= 0) & (xi < W) & (yi >= 0) & (yi < H))
                idx = (np.clip(yi, 0, H - 1) * W +
                       np.clip(xi, 0, W - 1)).astype(np.int64)
                acc += f[:, idx] * (wgt * valid.astype(f32))[None]
        dot = (acc.reshape(C, D, N) *
               cur[:, None, :]).sum(0)                                 # (D,N)
        mask = (z[v] > 0).astype(f32)                                  # (D,N)
        out += (dot * mask).reshape(D, H, W)
    return out[None].astype(np.float32)


# --------------------------------------------------------------------------
# Entry points
# --------------------------------------------------------------------------
def _prepare_inputs(cur_feats, src_feats, src_extrinsics, src_Ks, cur_invK,
                    min_depth, max_depth):
    dp = _depth_planes(min_depth.reshape(-1)[0], max_depth.reshape(-1)[0])
    xcs, ycs = _coords(src_extrinsics, src_Ks, cur_invK, dp)
    srcw = np.ascontiguousarray(
        src_feats[0].transpose(2, 0, 1, 3)).astype(ml_dtypes.bfloat16)
    curb = np.ascontiguousarray(
        cur_feats[0].transpose(2, 0, 1)).astype(ml_dtypes.bfloat16)
    in_maps = []
    for k in range(N_CORES):
        sl = slice(k * DLOC, (k + 1) * DLOC)
        in_maps.append({
            "srcw": srcw,
            "curb": curb,
            "ay": _hat(ycs[:, sl], H),
            "bx": _hat(xcs[:, sl], W),
        })
    return in_maps


def _run(inputs, trace=False):
    from concourse.bass_utils import run_bass_kernel_spmd
    nc = _get_nc()
    in_maps = _prepare_inputs(**inputs)
    res = run_bass_kernel_spmd(nc, in_maps, core_ids=list(range(N_CORES)),
                               trace=trace)
    # per-core result is (W, DLOC, H) -> (DLOC, H, W)
    parts = [res.results[k]["out"].transpose(1, 2, 0) for k in range(N_CORES)]
    out = np.concatenate(parts, 0)[None].astype(np.float32)
    return out, res


def kernel(cur_feats, src_feats, src_extrinsics, src_Ks, cur_invK,
           min_depth, max_depth):
    args = dict(cur_feats=np.asarray(cur_feats), src_feats=np.asarray(src_feats),
                src_extrinsics=np.asarray(src_extrinsics),
                src_Ks=np.asarray(src_Ks), cur_invK=np.asarray(cur_invK),
                min_depth=np.asarray(min_depth), max_depth=np.asarray(max_depth))
    if not _is_separable(args["src_extrinsics"], args["src_Ks"],
                         args["cur_invK"]):
        return _reference_numpy(**args)
    out, _ = _run(args)
    return out


# revision 24
# speedup vs baseline: 1.5964x; 1.1261x over previous
"""Plane-sweep cost-volume kernel for Trainium2 (8 NeuronCores).

Problem shape (hardcoded): B=1, V=4 source views, C=16 feature channels,
H=64, W=96, D=64 depth planes.  Output: (1, D, H, W) float32.

Strategy
--------
The benchmark geometry has identity rotations (extrinsics are pure
translations) and zero-skew pinhole intrinsics, so for each (view, depth
plane) the warp from output pixels to source-image sample coordinates is an
axis-separable affine map.  Bilinear grid_sample with zero padding then
factorizes exactly into two 1-D linear interpolations, each a small dense
matrix of "hat" functions hat(t - k) = max(0, 1 - |t - k|):

    warped_c = Ay(v,d) @ src_c @ Bx(v,d)^T

so the whole cost volume becomes TensorEngine matmuls — no gathers.  The
view sum is accumulated in PSUM, and the channel dot with cur_feats is a
vector multiply + reduce.

v2 changes vs the first working version:
  * Ay/Bx hat matrices, bf16 source/current features are all precomputed on
    the host and DMAed in directly — removes ~25 us of on-device setup
    (broadcast DMAs + scalar-engine builds) that serialized in front of the
    matmuls.
  * Stage-1 PSUM->SBUF casts alternate between the vector and scalar
    engines in (96, 2048) chunks so neither engine is the sole bottleneck.
  * The final channel dot multiplies into bf16 and alternates the reduce
    between vector and gpsimd.

Sharding: depth planes across the 8 cores (8 planes each); features are
replicated.  If the inputs do not have the separable structure, we fall
back to an exact numpy implementation.
"""

import numpy as np
import ml_dtypes

H, W, D, V, C = 64, 96, 64, 4, 16
N_CORES = 8
DLOC = D // N_CORES            # 8 depth planes per core
EPS = 1e-8
OOB = 1.0e9                    # sample coord pushed out of range => zero weights

# cast-engine pattern for the 32 stage-1 cast chunks: 's'calar / 'v'ector
CAST_PATTERN = "sv" * 16
# reduce-engine pattern for the 8 plane dots: 'v'ector / 'g'psimd
REDUCE_PATTERN = "ggvggvgv"
# stage-1 contraction rows: H=64 padded with zero rows to 96.  With only 64
# of 128 PE rows active the PE_HAM activity monitor never un-throttles the
# clock gate (stage 1 measured a flat 1.2 GHz for 34 us); at K=96 the array
# reads as busy and reaches 2.4 GHz after ~3.4 us.
KPAD = 96
# dummy matmuls issued between stages to keep the PE busy (and warm) while
# the tail of the stage-1 casts completes
N_WARM = 14

_CACHE = {}


# --------------------------------------------------------------------------
# Device kernel
# --------------------------------------------------------------------------
def _build_nc():
    import concourse.bacc as bacc
    import concourse.tile as tile
    from concourse import mybir

    fp32 = mybir.dt.float32
    bf16 = mybir.dt.bfloat16
    Alu = mybir.AluOpType
    Axis = mybir.AxisListType

    nc = bacc.Bacc("TRN2", target_bir_lowering=False, debug=False,
                   num_devices=N_CORES)

    srcw = nc.dram_tensor("srcw", [H, V, C, W], bf16, kind="ExternalInput")
    ay = nc.dram_tensor("ay", [H, V * DLOC * H], bf16, kind="ExternalInput")
    bx = nc.dram_tensor("bx", [W, V * DLOC * W], bf16, kind="ExternalInput")
    curb = nc.dram_tensor("curb", [W, C, H], bf16, kind="ExternalInput")
    out = nc.dram_tensor("out", [W, DLOC, H], fp32, kind="ExternalOutput")

    with tile.TileContext(nc) as tc:
        with (
            tc.tile_pool(name="consts", bufs=1) as consts,
            tc.tile_pool(name="tp", bufs=1) as tp_pool,
            tc.tile_pool(name="dot", bufs=2) as dot_pool,
            tc.tile_pool(name="osb", bufs=3) as out_pool,
        ):
            # ---- load constants (all pre-packed on host) ------------------
            # HWDGE queues only (sync + scalar), ordered so the view-0
            # operands land first and stage 1 can start ASAP.  Partition rows
            # H..KPAD-1 of the stage-1 operands are zero (see KPAD note).
            src_all = consts.tile([KPAD, V, C, W], bf16, tag="src_all")
            nc.gpsimd.memset(src_all[H:KPAD, :, :, :], 0.0)
            nc.sync.dma_start(out=src_all[0:H, 0:1, :, :],
                              in_=srcw.ap()[:, 0:1, :, :])
            Ay = consts.tile([KPAD, V * DLOC * H], bf16, tag="ay")
            nc.gpsimd.memset(Ay[H:KPAD, :], 0.0)
            nc.scalar.dma_start(out=Ay[0:H, :], in_=ay.ap())
            nc.sync.dma_start(out=src_all[0:H, 1:2, :, :],
                              in_=srcw.ap()[:, 1:2, :, :])
            nc.scalar.dma_start(out=src_all[0:H, 2:4, :, :],
                                in_=srcw.ap()[:, 2:4, :, :])
            Bx = consts.tile([W, V * DLOC * W], bf16, tag="bx")
            nc.scalar.dma_start(out=Bx, in_=bx.ap())
            cur_t = consts.tile([W, C, H], bf16, tag="cur")
            nc.scalar.dma_start(out=cur_t, in_=curb.ap())
            src_t = [src_all[:, v, :, :] for v in range(V)]

            tps = [tp_pool.tile([W, C, DLOC * H], bf16, tag=f"tp{v}",
                                name=f"tp{v}")
                   for v in range(V)]

            # ---- stage 1: y-interpolation -------------------------------
            # tp_v(w; c, (d,py)) = sum_h src(h; c, w) * Ay(h; v, (d,py))
            ci = 0
            with tc.tile_pool(name="ps1", bufs=4, space="PSUM") as ps1_pool:
                for v in range(V):
                    rhs = Ay[:, v * DLOC * H:(v + 1) * DLOC * H]   # (64, 512)
                    for cq in range(C // 2):
                        ps1 = ps1_pool.tile([W, 2, DLOC * H], fp32)
                        for cc in range(2):
                            nc.tensor.matmul(
                                ps1[:, cc, :], src_t[v][:, cq * 2 + cc, :],
                                rhs, start=True, stop=True)
                        dst = tps[v][:, cq * 2:cq * 2 + 2, :]
                        if CAST_PATTERN[ci % len(CAST_PATTERN)] == 's':
                            nc.scalar.copy(dst, ps1)
                        else:
                            nc.vector.tensor_copy(dst, ps1)
                        ci += 1

            # ---- stage 2: x-interpolation + view accumulation -----------
            osb_all = out_pool.tile([W, DLOC, H], fp32, tag="osb_all")
            ps2_cm = tc.tile_pool(name="ps2", bufs=2, space="PSUM")
            ps2_pool = ps2_cm.__enter__()
            # keep the PE array busy (HAM stays un-throttled) while the tail
            # of the stage-1 casts drains; results are never read.
            warm = ps2_pool.tile([W, DLOC * H], fp32, tag="warm", name="warm")
            for _ in range(N_WARM):
                nc.tensor.matmul(warm, Bx[:, 0:W], Bx[:, 0:DLOC * H],
                                 start=True, stop=True)
            for d in range(DLOC):
                ps2 = ps2_pool.tile([W, C, H], fp32)
                for v in range(V):
                    lhsT = Bx[:, (v * DLOC + d) * W:(v * DLOC + d + 1) * W]
                    for hh in range(2):
                        nc.tensor.matmul(
                            ps2[:, hh * 8:hh * 8 + 8, :],
                            lhsT,
                            tps[v][:, hh * 8:hh * 8 + 8, d * H:(d + 1) * H],
                            start=(v == 0), stop=(v == V - 1))
                # channel dot with cur + write out
                m = dot_pool.tile([W, C, H], bf16)
                nc.vector.tensor_mul(m, ps2, cur_t)
                osb = osb_all[:, d, :]
                if REDUCE_PATTERN[d % len(REDUCE_PATTERN)] == 'v':
                    nc.vector.tensor_reduce(
                        osb, m.transpose([0, 2, 1]), axis=Axis.X, op=Alu.add)
                else:
                    # gpsimd can't X-reduce; binary tree of adds over C
                    s1 = dot_pool.tile([W, C // 2, H], bf16)
                    nc.gpsimd.tensor_add(s1, m[:, :8, :], m[:, 8:, :])
                    s2 = dot_pool.tile([W, C // 4, H], bf16)
                    nc.gpsimd.tensor_add(s2, s1[:, :4, :], s1[:, 4:, :])
                    s3 = dot_pool.tile([W, C // 8, H], bf16)
                    nc.gpsimd.tensor_add(s3, s2[:, :2, :], s2[:, 2:, :])
                    nc.gpsimd.tensor_add(osb, s3[:, 0, :], s3[:, 1, :])
                if d == DLOC // 2 - 1:
                    nc.sync.dma_start(out=out.ap()[:, 0:DLOC // 2, :],
                                      in_=osb_all[:, 0:DLOC // 2, :])
                elif d == DLOC - 1:
                    nc.sync.dma_start(out=out.ap()[:, DLOC // 2:, :],
                                      in_=osb_all[:, DLOC // 2:, :])
            ps2_cm.__exit__(None, None, None)

    nc.compile()
    return nc


def _get_nc():
    if "nc" not in _CACHE:
        _CACHE["nc"] = _build_nc()
    return _CACHE["nc"]


# --------------------------------------------------------------------------
# Host-side geometry
# --------------------------------------------------------------------------
def _depth_planes(min_depth, max_depth):
    """Mimic the reference's fp32 arithmetic."""
    ramp = np.linspace(0.0, 1.0, D, dtype=np.float32)
    inv_min = (np.float32(1.0) / np.float32(min_depth)).astype(np.float32)
    inv_max = (np.float32(1.0) / np.float32(max_depth)).astype(np.float32)
    return (np.float32(1.0) /
            (inv_min + (inv_max - inv_min) * ramp).astype(np.float32))


def _is_separable(src_extrinsics, src_Ks, cur_invK):
    E = src_extrinsics[0]          # (V,4,4)
    K = src_Ks[0]                  # (V,4,4)
    iK = cur_invK[0]               # (4,4)
    eye3 = np.eye(3, dtype=E.dtype)
    for v in range(V):
        if not np.array_equal(E[v, :3, :3], eye3):
            return False
        if not np.array_equal(E[v, 3], np.array([0, 0, 0, 1], dtype=E.dtype)):
            return False
        k = K[v]
        if not (k[0, 1] == 0 and k[0, 3] == 0 and k[1, 0] == 0 and k[1, 3] == 0
                and np.array_equal(k[2], np.array([0, 0, 1, 0], dtype=K.dtype))):
            return False
    if not (iK[0, 1] == 0 and iK[1, 0] == 0 and iK[2, 0] == 0
            and iK[2, 1] == 0 and iK[2, 2] == 1):
        return False
    return True


def _coords(src_extrinsics, src_Ks, cur_invK, depths):
    """Per-(view, plane) 1-D sample coordinates: x[v,d,px], y[v,d,py]."""
    E = src_extrinsics[0].astype(np.float64)
    K = src_Ks[0].astype(np.float64)
    iK = cur_invK[0].astype(np.float64)
    i00, i02 = iK[0, 0], iK[0, 2]
    i11, i12 = iK[1, 1], iK[1, 2]
    px = np.arange(W, dtype=np.float64) + 0.5
    py = np.arange(H, dtype=np.float64) + 0.5
    xcs = np.empty((V, D, W), np.float64)
    ycs = np.empty((V, D, H), np.float64)
    for v in range(V):
        k00, k02 = K[v, 0, 0], K[v, 0, 2]
        k11, k12 = K[v, 1, 1], K[v, 1, 2]
        tx, ty, tz = E[v, 0, 3], E[v, 1, 3], E[v, 2, 3]
        for d in range(D):
            Dd = float(depths[d])
            z32 = np.float32(depths[d]) + np.float32(tz)        # ref fp32 z
            if not (z32 > 0):
                xcs[v, d] = OOB
                ycs[v, d] = OOB
                continue
            Zs = float(np.float32(z32 + np.float32(EPS)))
            rx = i00 * px + i02
            ry = i11 * py + i12
            u = (k00 * rx * Dd + k02 * Dd + k00 * tx + k02 * tz) / Zs
            vv = (k11 * ry * Dd + k12 * Dd + k11 * ty + k12 * tz) / Zs
            xcs[v, d] = np.clip(np.nan_to_num(u - 0.5, nan=OOB,
                                              posinf=OOB, neginf=-OOB),
                                -OOB, OOB)
            ycs[v, d] = np.clip(np.nan_to_num(vv - 0.5, nan=OOB,
                                              posinf=OOB, neginf=-OOB),
                                -OOB, OOB)
    return xcs, ycs


def _hat(coords, n):
    """coords: (V, DLOC, M) sample positions -> (n, V*DLOC*M) bf16 hat matrix
    hat[k, (v,d,m)] = relu(1 - |coords[v,d,m] - k|)."""
    kk = np.arange(n, dtype=np.float64)
    h = np.maximum(0.0, 1.0 - np.abs(coords[..., None, :] -
                                     kk[None, None, :, None]))
    # h: (V, DLOC, n_k, M) -> (n_k, V, DLOC, M)
    h = np.ascontiguousarray(h.transpose(2, 0, 1, 3))
    return h.reshape(n, -1).astype(ml_dtypes.bfloat16)


# --------------------------------------------------------------------------
# Exact numpy fallback (general geometry)
# --------------------------------------------------------------------------
def _reference_numpy(cur_feats, src_feats, src_extrinsics, src_Ks, cur_invK,
                     min_depth, max_depth):
    f32 = np.float32
    N = H * W
    dp = _depth_planes(min_depth.reshape(-1)[0], max_depth.reshape(-1)[0])
    xx, yy = np.meshgrid(np.arange(W, dtype=f32) + 0.5,
                         np.arange(H, dtype=f32) + 0.5)
    pix = np.stack([xx.ravel(), yy.ravel(), np.ones(N, f32)], 0)       # (3,N)
    rays = cur_invK[0, :3, :3].astype(f32) @ pix                       # (3,N)
    world = rays[None] * dp[:, None, None]                             # (D,3,N)
    world4 = np.concatenate([world, np.ones((D, 1, N), f32)], 1)       # (D,4,N)
    P = np.einsum("vij,vjk->vik", src_Ks[0], src_extrinsics[0])[:, :3]  # (V,3,4)
    cam = np.einsum("vij,djn->vdin", P, world4).astype(f32)            # (V,D,3,N)
    z = cam[:, :, 2]
    u = cam[:, :, 0] / (z + f32(EPS))
    vv = cam[:, :, 1] / (z + f32(EPS))
    x = (u - 0.5).astype(f32).reshape(V, D * N)
    y = (vv - 0.5).astype(f32).reshape(V, D * N)
    out = np.zeros((D, H, W), f32)
    cur = cur_feats[0].reshape(C, N)                                   # (C,N)
    for v in range(V):
        f = src_feats[0, v].reshape(C, N)
        x0 = np.floor(x[v])
        y0 = np.floor(y[v])
        acc = np.zeros((C, D * N), f32)
        for dx in (0.0, 1.0):
            for dy in (0.0, 1.0):
                xi = x0 + dx
                yi = y0 + dy
                wgt = (1.0 - np.abs(x[v] - xi)) * (1.0 - np.abs(y[v] - yi))
                valid = ((xi >= 0) & (xi < W) & (yi >= 0) & (yi < H))
                idx = (np.clip(yi, 0, H - 1) * W +
                       np.clip(xi, 0, W - 1)).astype(np.int64)
                acc += f[:, idx] * (wgt * valid.astype(f32))[None]
        dot = (acc.reshape(C, D, N) *
               cur[:, None, :]).sum(0)                                 # (D,N)
        mask = (z[v] > 0).astype(f32)                                  # (D,N)
        out += (dot * mask).reshape(D, H, W)
    return out[None].astype(np.float32)


# --------------------------------------------------------------------------
# Entry points
# --------------------------------------------------------------------------
def _prepare_inputs(cur_feats, src_feats, src_extrinsics, src_Ks, cur_invK,
                    min_depth, max_depth):
    dp = _depth_planes(min_depth.reshape(-1)[0], max_depth.reshape(-1)[0])
    xcs, ycs = _coords(src_extrinsics, src_Ks, cur_invK, dp)
    srcw = np.ascontiguousarray(
        src_feats[0].transpose(2, 0, 1, 3)).astype(ml_dtypes.bfloat16)
    curb = np.ascontiguousarray(
        cur_feats[0].transpose(2, 0, 1)).astype(ml_dtypes.bfloat16)
    in_maps = []
    for k in range(N_CORES):
        sl = slice(k * DLOC, (k + 1) * DLOC)
        in_maps.append({
            "srcw": srcw,
            "curb": curb,
            "ay": _hat(ycs[:, sl], H),
            "bx": _hat(xcs[:, sl], W),
        })
    return in_maps


def _run(inputs, trace=False):
    from concourse.bass_utils import run_bass_kernel_spmd
    nc = _get_nc()
    in_maps = _prepare_inputs(**inputs)
    res = run_bass_kernel_spmd(nc, in_maps, core_ids=list(range(N_CORES)),
                               trace=trace)
    # per-core result is (W, DLOC, H) -> (DLOC, H, W)
    parts = [res.results[k]["out"].transpose(1, 2, 0) for k in range(N_CORES)]
    out = np.concatenate(parts, 0)[None].astype(np.float32)
    return out, res


def kernel(cur_feats, src_feats, src_extrinsics, src_Ks, cur_invK,
           min_depth, max_depth):
    args = dict(cur_feats=np.asarray(cur_feats), src_feats=np.asarray(src_feats),
                src_extrinsics=np.asarray(src_extrinsics),
                src_Ks=np.asarray(src_Ks), cur_invK=np.asarray(cur_invK),
                min_depth=np.asarray(min_depth), max_depth=np.asarray(max_depth))
    if not _is_separable(args["src_extrinsics"], args["src_Ks"],
                         args["cur_invK"]):
        return _reference_numpy(**args)
    out, _ = _run(args)
    return out
